# revision 5
# baseline (speedup 1.0000x reference)
"""Trainium2 Bass kernel for nn_ContrastiveLoss (SimCLR-style NT-Xent loss).

Reference computation:
    f = normalize(concat([z1, z2]))            # [2B, D] unit rows
    S = f @ f.T / T                            # [8192, 8192]
    loss = mean_i( logsumexp_j(S[i, :]) - S[i, pos_i] )

Sharding: each of the 8 cores owns a 1024-row block of S and computes it
against all 8192 columns. To keep one SPMD program for all cores, each
core receives the 8 row-groups of F = [z1; z2] ROTATED so its own rows
are always group 0 (the loss is invariant to the column permutation, and
rolling by a multiple of B keeps pos offsets at +4096).

Device-side plan per core:
  1. Cast-load each group's 1024 rows as [128, 8, 512] bf16 (gpsimd DMA
     cast), per-row sum-of-squares (DVE affine_mul_reduce), Quake-style
     Newton rsqrt on DVE, then scale rows to unit norm with the output
     cast straight to fp8e4 in a [p][kh][row-tile][256] layout.
  2. Transpose each (group, k-half) with ONE SBUF->SBUF DMA-xbar
     instruction, treating adjacent fp8 pairs as uint16 units. The
     result [128, 1024] lands exactly in the [128dp, 2pair, cols] layout
     the fp8 DoubleRow matmul wants (contraction d = 256*kh + 2*dp + pair
     consistently on both operands).
  3. fp8e4 DoubleRow matmuls (contraction 256/instr, 2 elem/cycle) build
     [128, 2048] row-blocks of cos-sim in PSUM; ACT computes exp(x/T)
     in place with a fused row-sum (accum_out).
  4. The diagonal (group 0) and positive-pair (group 4) cosines are
     pulled off PSUM pre-exp with an identity-mask mul-reduce on DVE --
     no separate zme/zpos loads.

Outputs per core: "sums" [128, 32] (exp-sums per row over 2048-col
chunks), "diag"/"pos" [128, 8] (raw fp8-precision cosines). Host (f64):
  R_i  = sum(sums_i) - exp(diag_i/T) + exp(1/T)    # exact-diagonal fix
  loss = mean(log(R_i) - pos_i/T)
The diagonal substitution is exact (unit rows have cos=1 analytically)
and cancels the fp8 quantization noise of the dominant softmax term.

The max-subtraction in the reference logsumexp is skipped on device:
|cos|/T <= ~14.3, so sum_j exp() <= ~2e10, well within fp32 range.
"""

import os
import sys

try:
    import concourse.bass  # noqa: F401
except ImportError:
    for _p in ("/root/.axon_site/_ro/trn_rl_repo", "/opt/trn_rl_repo"):
        if _p not in sys.path and os.path.isdir(_p):
            sys.path.insert(0, _p)

import numpy as np

B = 4096
D = 512
T = 0.07
P = 128
NCORES = 8
R = (2 * B) // NCORES  # 1024 rows per core / per group
G = (2 * B) // R       # 8 column groups
GT = R // P            # 8 row tiles per group
H = 2                  # contraction halves (256 each) for DoubleRow
PSW = 2048             # psum tile width (4 banks)
NB = (2 * B) // PSW    # 4 psum tiles per row tile
NS = PSW // 512        # 4 matmul n-slices per psum tile

_NC = None


def _build():
    from contextlib import ExitStack

    import concourse.bacc as bacc
    import concourse.tile as tile
    from concourse import mybir

    f32 = mybir.dt.float32
    bf16 = mybir.dt.bfloat16
    f8 = mybir.dt.float8e4
    u16 = mybir.dt.uint16
    i32 = mybir.dt.int32
    AFT = mybir.ActivationFunctionType
    EXPF = AFT.Exp
    MUL = mybir.AluOpType.mult
    ADD = mybir.AluOpType.add
    SUB = mybir.AluOpType.subtract
    SHR = mybir.AluOpType.logical_shift_right
    DR = mybir.MatmulPerfMode.DoubleRow

    nc = bacc.Bacc(
        "TRN2", target_bir_lowering=False, debug=False, num_devices=NCORES
    )
    fg = [
        nc.dram_tensor(f"f{k}", [R, D], f32, kind="ExternalInput")
        for k in range(G)
    ]
    eye = nc.dram_tensor("eye", [P, P], f32, kind="ExternalInput")
    sums_out = nc.dram_tensor("sums", [P, GT * NB], f32, kind="ExternalOutput")
    diag_out = nc.dram_tensor("diag", [P, GT], f32, kind="ExternalOutput")
    pos_out = nc.dram_tensor("pos", [P, GT], f32, kind="ExternalOutput")

    with ExitStack() as ctx:
        tc = ctx.enter_context(tile.TileContext(nc))
        smalls = ctx.enter_context(tc.tile_pool(name="smalls", bufs=1))
        dumps = ctx.enter_context(tc.tile_pool(name="dumps", bufs=4))
        stats = ctx.enter_context(tc.tile_pool(name="stats", bufs=3))
        zbpool = ctx.enter_context(tc.tile_pool(name="zbpool", bufs=3))
        fnpool = ctx.enter_context(tc.tile_pool(name="fnpool", bufs=2))
        ftpool = ctx.enter_context(tc.tile_pool(name="ftpool", bufs=1))
        psum = ctx.enter_context(tc.tile_pool(name="psum", bufs=2, space="PSUM"))

        sums_sb = smalls.tile([P, GT * NB], f32, tag="sums_sb")
        diag_sb = smalls.tile([P, GT], f32, tag="diag_sb")
        pos_sb = smalls.tile([P, GT], f32, tag="pos_sb")
        eye_sb = smalls.tile([P, P], f32, tag="eye_sb")
        nc.sync.dma_start(out=eye_sb[:], in_=eye[:, :])
        magic = smalls.tile([P, GT], i32, tag="magic")
        nc.vector.memset(magic[:], 0x5F3759DF)

        def mulsum(in0, in1, accum_col):
            # accum_col[p] = sum_x in0[p,x]*in1[p,x] in one DVE op; the
            # mandatory main output goes to a throwaway broadcast AP.
            dummy = dumps.tile([P, 1], f32, tag="dummy")
            nc.vector.affine_mul_reduce(
                out=dummy.broadcast_to(in0.shape),
                accum_out=accum_col,
                in0=in0,
                in1=in1,
                scale=1.0,
                bias=0.0,
            )

        def rsqrt(invn_dst, ssq):
            # 1/max(sqrt(s), eps) == min(rsqrt(s), 1e12); Quake bit-trick
            # + 2 Newton iterations, all on DVE -- keeps ACT's table set
            # pinned to Exp for the whole kernel.
            n = ssq.shape[1]
            h = stats.tile([P, n], i32, tag="h")
            nc.vector.tensor_scalar(h[:], ssq.bitcast(i32), 1, None, op0=SHR)
            y = stats.tile([P, n], f32, tag="y")
            nc.vector.tensor_tensor(y[:].bitcast(i32), magic[:, :n], h[:], op=SUB)
            a = stats.tile([P, n], f32, tag="a")
            for _ in range(2):
                nc.vector.tensor_mul(a[:], y[:], y[:])
                nc.vector.tensor_mul(a[:], a[:], ssq)
                nc.vector.tensor_scalar(a[:], a[:], -0.5, 1.5, op0=MUL, op1=ADD)
                nc.vector.tensor_mul(y[:], y[:], a[:])
            nc.vector.tensor_scalar_min(invn_dst, y[:], 1.0e12)

        ft2 = [[None] * H for _ in range(G)]
        # Stationary-operand copy of group 0: the DoubleRow LDWEIGHTS ISA
        # check (s3_lw_dual_fp8_restrictions) requires the k-pair dim to
        # have stride %16 elements, so the byte-interleaved ft2 layout is
        # legal only for the moving operand. Deinterleave own rows once:
        # w8 [dp][h][pair][j] with pair stride R.
        w8 = smalls.tile([P, H, 2, R], f8, tag="w8")

        def build_group(g):
            zb = zbpool.tile([P, GT, D], bf16, tag="zb")
            for s in range(2):
                nc.gpsimd.dma_start(
                    out=zb[:, s * 4 : (s + 1) * 4, :],
                    in_=fg[g][s * 4 * P : (s + 1) * 4 * P, :].rearrange(
                        "(a p) d -> p a d", p=P
                    ),
                )
            ssq = stats.tile([P, GT], f32, tag="ssq")
            for a in range(GT):
                mulsum(zb[:, a, :], zb[:, a, :], ssq[:, a : a + 1])
            invn = stats.tile([P, GT], f32, tag="invn")
            rsqrt(invn[:], ssq[:])
            # fn8 layout: [p][kh][row-tile][256] so each k-half is a
            # contiguous [128, 2048] fp8 = [128, 1024] u16 transpose src.
            fn8 = fnpool.tile([P, H, GT, D // H], f8, tag="fn8")
            for a in range(GT):
                nc.vector.tensor_scalar_mul(
                    fn8[:, :, a, :],
                    zb[:, a, :].rearrange("p (h x) -> p h x", h=H),
                    invn[:, a : a + 1],
                )
            for h in range(H):
                fth = ftpool.tile([P, R], u16, tag=f"ft{g}_{h}", name=f"ft{g}_{h}")
                nc.sync.dma_start(
                    out=fth[:].rearrange("p (a j) -> p a j", a=GT),
                    in_=fn8[:, h].bitcast(u16),
                    transpose=True,
                )
                ft2[g][h] = fth
                if g == 0:
                    nc.vector.tensor_copy(w8[:, h], f8view(0, h))

        def f8view(g, h):
            # [128, 1024] u16 -> [128 dp, 2 pair, 1024 cols] fp8; the
            # contraction index is d = 256*h + 2*dp + pair on BOTH sides.
            return ft2[g][h][:].bitcast(f8).rearrange(
                "p (j two) -> p two j", two=2
            )

        def sim_block(nb):
            for r in range(GT):
                ps = psum.tile([P, PSW], f32, tag="ps")
                for h in range(H):
                    lhsT = w8[:, h, :, r * P : (r + 1) * P]
                    for ns in range(NS):
                        j0 = nb * PSW + ns * 512
                        gj, cj = divmod(j0, R)
                        nc.tensor.matmul(
                            ps[:, ns * 512 : (ns + 1) * 512],
                            lhsT,
                            f8view(gj, h)[:, :, cj : cj + 512],
                            start=(h == 0),
                            stop=(h == H - 1),
                            perf_mode=DR,
                        )
                # Raw-cosine extraction must read PSUM before the in-place
                # exp. Own rows are group 0, pos pairs group 4 (rolled).
                if nb == 0:
                    mulsum(ps[:, r * P : (r + 1) * P], eye_sb[:], diag_sb[:, r : r + 1])
                if nb == 2:
                    mulsum(ps[:, r * P : (r + 1) * P], eye_sb[:], pos_sb[:, r : r + 1])
                idx = r * NB + nb
                nc.scalar.activation(
                    ps[:],
                    ps[:],
                    EXPF,
                    scale=1.0 / T,
                    accum_out=sums_sb[:, idx : idx + 1],
                )

        # Program order doubles as scheduler priority: the two groups a
        # column-block needs are built right before its matmuls; later
        # groups' loads gap-fill under PE/ACT work.
        for nb in range(NB):
            build_group(2 * nb)
            build_group(2 * nb + 1)
            sim_block(nb)

        nc.sync.dma_start(out=sums_out[:], in_=sums_sb[:])
        nc.sync.dma_start(out=diag_out[:], in_=diag_sb[:])
        nc.sync.dma_start(out=pos_out[:], in_=pos_sb[:])

    nc.compile()
    return nc


def _get_nc():
    global _NC
    if _NC is None:
        _NC = _build()
    return _NC


def run(z1, z2, trace=False):
    """Run the SPMD kernel; returns (loss, BassKernelResults)."""
    from concourse.bass_utils import run_bass_kernel_spmd

    z1 = np.ascontiguousarray(z1, dtype=np.float32)
    z2 = np.ascontiguousarray(z2, dtype=np.float32)
    F = np.concatenate([z1, z2], axis=0)  # [8192, 512]
    eye_np = np.eye(P, dtype=np.float32)
    in_maps = []
    for c in range(NCORES):
        m = {"eye": eye_np}
        for k in range(G):
            blk = (c + k) % G
            m[f"f{k}"] = F[blk * R : (blk + 1) * R]
        in_maps.append(m)
    res = run_bass_kernel_spmd(
        _get_nc(), in_maps, core_ids=list(range(NCORES)), trace=trace
    )
    e_diag_true = np.exp(1.0 / T)
    total = 0.0
    for r in res.results:
        sums = r["sums"].astype(np.float64)  # [P, GT*NB]
        diag = r["diag"].astype(np.float64)  # [P, GT] own-cos (~1 + fp8 noise)
        pos = r["pos"].astype(np.float64)    # [P, GT] positive-pair cosines
        sumexp = sums.reshape(P, GT, NB).sum(axis=2)
        sumexp = sumexp - np.exp(diag / T) + e_diag_true
        total += (np.log(sumexp) - pos / T).sum()
    loss = total / (2.0 * B)
    return np.float32(loss), res


def kernel(z1, z2, labels=None, **_ignored):
    loss, _ = run(z1, z2, trace=False)
    return np.asarray(loss, dtype=np.float32)


if __name__ == "__main__":
    rng = np.random.default_rng(0)
    a = rng.standard_normal((B, D)).astype(np.float32)
    b = rng.standard_normal((B, D)).astype(np.float32)
    print(kernel(a, b, None))
